# revision 2
# baseline (speedup 1.0000x reference)
"""Trainium2 Bass kernel for nn_CrossAttention (B=8, N=M=2048, C=512, H=4).

Sharding: data-parallel over batch — one batch element per NeuronCore (8 cores).
Per-core dataflow (every matmul contracts over the partition dim; fp16 operands
everywhere -> 1-cycle/row PE; fp32 PSUM accumulation):

  1. Fused input pipeline, per 512-row stripe (F2 first, then F1):
       casting DMA (gpsimd SWDGE) loads the stripe fp32->fp16 directly,
       16 fp16 PE transposes -> F^T chunks, then the qkv projection matmuls
       for that stripe (lhsT = W chunk, rhs = F^T stripe). Bias fused into
       the PSUM->SBUF evac as tensor_scalar_add with a per-partition column.
     This keeps PE busy during the HBM-bound input phase.
  2. kv natural (m-major) via fp16 PE transposes of kv^T blocks.
  3. Attention per (head, n-stripe of 512):
       scores^T[m,n] = kv_h^T.T @ q_h^T   (PSUM, 2 banks per pair of m-blocks)
       E^T = exp(SCALE * scores^T)        (ACT, PSUM->SBUF, fp16; no max-sub:
                                           |SCALE*s| <= ~2, exp safe in fp32)
       unnorm^T[d,n] = sum_m kv_h-blocks.T @ E^T   (PE, fp16 in, f32 acc)
       denominators:  DVE sums E block pairs in place (E[2j] += E[2j+1]),
                      then 8 PE ones-matmuls (not 16) reduce over partitions
       recip = 1/denom on the [1,n] row (DVE); partition-broadcast on GPSIMD
       x^T[d,n] = unnorm^T * bcast(recip)  (DVE mul, writes fp16)
  4. out[n,c] = x^T-blocks.T @ W_proj (PE) + b_proj via DVE tensor_add with a
     pre-broadcast bias tile (no rank-1 bias matmul, no ACT copy), DMA out.
"""
import sys

for _p in ("/opt/trn_rl_repo", "/root/.axon_site/_ro/trn_rl_repo"):
    if _p not in sys.path:
        sys.path.insert(0, _p)

import numpy as np
import concourse.bass as bass
import concourse.bacc as bacc
import concourse.tile as tile
from concourse import mybir
from concourse.bass_utils import run_bass_kernel_spmd

F32 = mybir.dt.float32
F16 = mybir.dt.float16
EXP = mybir.ActivationFunctionType.Exp

B, N, M, C = 8, 2048, 2048, 512
H, D = 4, 128
SCALE = 1.0 / np.sqrt(C)
P = 128
NB = N // P        # 16 n-blocks
MB = M // P        # 16 m-blocks
KC = C // P        # 4 contraction chunks (also = heads since D=128)
NS = 4             # n-stripes of 512
SW = N // NS       # stripe width 512


def build_nc():
    nc = bacc.Bacc(None, target_bir_lowering=False)
    dF1 = nc.dram_tensor("F1", [N, C], F32, kind="ExternalInput")
    dF2 = nc.dram_tensor("F2", [M, C], F32, kind="ExternalInput")
    dW = nc.dram_tensor("Wqkv", [C, C], F32, kind="ExternalInput")
    dBq = nc.dram_tensor("bqkv", [1, C], F32, kind="ExternalInput")
    dWp = nc.dram_tensor("Wproj", [C, C], F32, kind="ExternalInput")
    dBp = nc.dram_tensor("bproj", [1, C], F32, kind="ExternalInput")
    dOut = nc.dram_tensor("OUT", [N, C], F32, kind="ExternalOutput")

    d_ident16 = nc.inline_tensor(np.eye(P, dtype=np.float16), name="identity16")
    d_ones_col = nc.inline_tensor(np.ones((P, 1), np.float16), name="ones_col")

    with tile.TileContext(nc) as tc:
        with (
            tc.tile_pool(name="const", bufs=1) as const,
            tc.tile_pool(name="persist", bufs=1) as persist,
        ):
            # ---- small constants first (cheap, needed early) ----
            ident16 = const.tile([P, P], F16)
            nc.sync.dma_start(ident16, d_ident16[:])
            ones_col = const.tile([P, 1], F16)
            nc.sync.dma_start(ones_col, d_ones_col[:])
            bq_col = const.tile([P, KC], F32)
            nc.sync.dma_start(
                bq_col, dBq[0, :].rearrange("(a b) -> b a", b=P)
            )

            # ---- persistent activations (kc-major packed single tiles) ----
            FT = persist.tile([P, KC, N], F16)    # F^T, reused F2 then F1
            qT = persist.tile([P, KC, N], F16)
            kvT = persist.tile([P, KC, M], F16)
            kvn = persist.tile([P, MB, C], F16)   # kv natural, m-major
            xT = persist.tile([P, KC, N], F16)
            W = []
            for kc in range(KC):
                w = const.tile([P, C], F16, name=f"w{kc}")
                W.append(w)

            # ---- phase 1+2 fused: per-stripe load -> transpose -> proj ----
            with (
                tc.tile_pool(name="fin", bufs=3) as fpool,
                tc.tile_pool(name="ps12", bufs=1, space="PSUM") as ps12,
            ):
                first_w = [True]

                def stripes(dsrc, dst):
                    for g in range(NS):
                        fin = fpool.tile([P, 4, C], F16, tag="fin")
                        nc.gpsimd.dma_start(
                            fin,
                            dsrc[g * SW:(g + 1) * SW, :].rearrange(
                                "(j p) c -> p j c", p=P
                            ),
                        )
                        if first_w[0]:
                            # W needed right after stripe 0 lands; emit its
                            # casting DMAs behind the first stripe's.
                            for kc in range(KC):
                                nc.gpsimd.dma_start(
                                    W[kc], dW[kc * P:(kc + 1) * P, :]
                                )
                            first_w[0] = False
                        for half in range(2):
                            tp = ps12.tile([P, 2, SW], F16, tag="tr", bufs=2)
                            for j in range(4):
                                for k in range(2):
                                    kc = 2 * half + k
                                    nc.tensor.transpose(
                                        tp[:, k, j * P:(j + 1) * P],
                                        fin[:, j, kc * P:(kc + 1) * P],
                                        ident16,
                                    )
                            nc.vector.tensor_copy(
                                FT[:, 2 * half:2 * half + 2,
                                   g * SW:(g + 1) * SW],
                                tp,
                            )
                        for co in range(KC):
                            pj = ps12.tile([P, SW], F32, tag="pj", bufs=4)
                            for kc in range(KC):
                                nc.tensor.matmul(
                                    pj,
                                    W[kc][:, co * P:(co + 1) * P],
                                    FT[:, kc, g * SW:(g + 1) * SW],
                                    start=(kc == 0),
                                    stop=(kc == KC - 1),
                                )
                            nc.vector.tensor_scalar_add(
                                dst[:, co, g * SW:(g + 1) * SW],
                                pj,
                                bq_col[:, co:co + 1],
                            )

                stripes(dF2, kvT)
                # kv natural: transpose kv^T blocks (bias already folded)
                for mb in range(MB):
                    pjt = ps12.tile([P, C], F16, tag="tr", bufs=2)
                    for hh in range(H):
                        nc.tensor.transpose(
                            pjt[:, hh * P:(hh + 1) * P],
                            kvT[:, hh, mb * P:(mb + 1) * P],
                            ident16,
                        )
                    nc.vector.tensor_copy(kvn[:, mb, :], pjt)
                stripes(dF1, qT)

            # late consts for phase 4 (emitted after input DMAs)
            Wp = []
            for kc in range(KC):
                wp = const.tile([P, C], F16, name=f"wp{kc}")
                nc.gpsimd.dma_start(wp, dWp[kc * P:(kc + 1) * P, :])
                Wp.append(wp)
            bp_row = const.tile([1, C], F32)
            nc.sync.dma_start(bp_row, dBp[:])
            bp_bc = const.tile([P, C], F32)
            nc.gpsimd.partition_broadcast(bp_bc, bp_row)

            # ---- phase 3: attention per (head, n-stripe) ----
            with (
                tc.tile_pool(name="et", bufs=2) as epool,
                tc.tile_pool(name="ps3", bufs=1, space="PSUM") as ps3,
                tc.tile_pool(name="sm", bufs=2) as sm,
            ):
                for h in range(H):
                    for s in range(NS):
                        E = epool.tile([P, MB, SW], F16, tag="E")
                        pv = ps3.tile([P, SW], F32, tag="pv", bufs=2)
                        dn = ps3.tile([1, SW], F32, tag="dn", bufs=2)

                        def pv_pair(u):
                            for mb in (2 * u, 2 * u + 1):
                                nc.tensor.matmul(
                                    pv,
                                    kvn[:, mb, h * P:(h + 1) * P],
                                    E[:, mb, :],
                                    start=(mb == 0),
                                    stop=(mb == MB - 1),
                                )
                            # after the pv reads, fold E[2u+1] into E[2u] so
                            # the denominator needs half the ones-matmuls
                            with nc.allow_low_precision(
                                reason="pair sums <=15; fp16 keeps 5e-4 rel"
                            ):
                                nc.vector.tensor_add(
                                    E[:, 2 * u, :], E[:, 2 * u, :],
                                    E[:, 2 * u + 1, :],
                                )

                        def dn_mm(u):
                            nc.tensor.matmul(
                                dn,
                                ones_col,
                                E[:, 2 * u, :],
                                start=(u == 0),
                                stop=(u == MB // 2 - 1),
                            )

                        for j in range(MB // 2):
                            sc = ps3.tile([P, 2, SW], F32, tag="sc", bufs=2)
                            for i in range(2):
                                mb = 2 * j + i
                                nc.tensor.matmul(
                                    sc[:, i, :],
                                    kvT[:, h, mb * P:(mb + 1) * P],
                                    qT[:, h, s * SW:(s + 1) * SW],
                                    start=True,
                                    stop=True,
                                )
                            # exp over both banks in one ACT instruction
                            nc.scalar.activation(
                                E[:, 2 * j:2 * j + 2, :].rearrange(
                                    "p a b -> p (a b)"
                                ),
                                sc.rearrange("p a b -> p (a b)"),
                                EXP,
                                scale=float(SCALE),
                            )
                            if j > 0:
                                pv_pair(j - 1)
                            if j > 1:
                                dn_mm(j - 2)
                        pv_pair(MB // 2 - 1)
                        dn_mm(MB // 2 - 2)
                        dn_mm(MB // 2 - 1)

                        rcp = sm.tile([1, SW], F32, tag="rcp")
                        nc.vector.reciprocal(rcp, dn)
                        rb = sm.tile([P, SW], F32, tag="rb")
                        nc.gpsimd.partition_broadcast(rb, rcp)
                        with nc.allow_low_precision(
                            reason="x values O(0.1); fp16 keeps 5e-4 rel"
                        ):
                            nc.vector.tensor_mul(
                                xT[:, h, s * SW:(s + 1) * SW], pv, rb
                            )

            # ---- phase 4: output projection ----
            with (
                tc.tile_pool(name="ps4", bufs=1, space="PSUM") as ps4,
                tc.tile_pool(name="osb", bufs=3) as osb,
            ):
                for nb in range(NB):
                    pr = ps4.tile([P, C], F32, tag="pr", bufs=4)
                    for kc in range(KC):
                        nc.tensor.matmul(
                            pr,
                            xT[:, kc, nb * P:(nb + 1) * P],
                            Wp[kc],
                            start=(kc == 0),
                            stop=(kc == KC - 1),
                        )
                    ot = osb.tile([P, C], F32, tag="ot")
                    nc.vector.tensor_add(ot, pr, bp_bc)
                    nc.sync.dma_start(dOut[nb * P:(nb + 1) * P, :], ot)

    nc.compile()
    return nc


_NC = None


def _get_nc():
    global _NC
    if _NC is None:
        _NC = build_nc()
    return _NC


def kernel(F1, F2, W_qkv, b_qkv, W_proj, b_proj, _trace=False):
    F1 = np.ascontiguousarray(np.asarray(F1, dtype=np.float32))
    F2 = np.ascontiguousarray(np.asarray(F2, dtype=np.float32))
    W = np.ascontiguousarray(np.asarray(W_qkv, dtype=np.float32))
    bq = np.ascontiguousarray(np.asarray(b_qkv, dtype=np.float32)).reshape(1, C)
    Wpj = np.ascontiguousarray(np.asarray(W_proj, dtype=np.float32))
    bp = np.ascontiguousarray(np.asarray(b_proj, dtype=np.float32)).reshape(1, C)

    nc = _get_nc()
    in_maps = [
        {"F1": F1[b], "F2": F2[b], "Wqkv": W, "bqkv": bq, "Wproj": Wpj, "bproj": bp}
        for b in range(B)
    ]
    res = run_bass_kernel_spmd(
        nc, in_maps, core_ids=list(range(B)), trace=_trace
    )
    out = np.stack([res.results[b]["OUT"] for b in range(B)], axis=0)
    if _trace:
        return out, res
    return out


# revision 3
# speedup vs baseline: 1.5300x; 1.5300x over previous
"""Trainium2 Bass kernel for nn_CrossAttention (B=8, N=M=2048, C=512, H=4).

Sharding: data-parallel over batch — one batch element per NeuronCore (8 cores).
Per-core dataflow (every matmul contracts over the partition dim; fp16 operands
everywhere -> 1-cycle/row PE; fp32 PSUM accumulation):

  1. Fused input pipeline, software-pipelined per 512-row stripe (F2 then F1):
       casting DMA (gpsimd SWDGE) loads stripe g+2 fp32->fp16 while PE runs
       16 fp16 transposes of stripe g and the projection matmuls of stripe
       g-1 (lhsT = W chunk, rhs = F^T stripe; bias fused into the PSUM->SBUF
       evac as tensor_scalar_add with a per-partition column).
  2. kv natural (m-major) via fp16 PE transposes of kv^T blocks.
  3. Attention per (head, n-stripe of 512), j = pair of m-blocks:
       scores^T[m,n] = kv_h^T.T @ q_h^T   (PSUM, 2 banks per pair)
       E^T = exp(SCALE * scores^T)        (ACT, PSUM->SBUF, fp16; no max-sub:
                                           |SCALE*s| <= ~2, exp safe in fp32)
       unnorm^T[d,n] = sum_m kv_h-blocks.T @ E^T   (PE, fp16 in, f32 acc)
       denominators:  DVE sums E block pairs into D2 (grouped 2 pairs/instr),
                      then 8 PE ones-matmuls (not 16) reduce over partitions;
                      both staggered so PE never waits on the DVE queue
       normalization (recip on the [1,n] row + gpsimd bcast + DVE mul) is
       DEFERRED into the middle of the next (h,s) tile so the slow reciprocal
       never sits ahead of pair-adds in DVE's in-order queue
  4. out[n,c] = x^T-blocks.T @ W_proj (PE) + b_proj via DVE tensor_add with a
     pre-broadcast bias tile (no rank-1 bias matmul, no ACT copy), DMA out.
"""
import sys

for _p in ("/opt/trn_rl_repo", "/root/.axon_site/_ro/trn_rl_repo"):
    if _p not in sys.path:
        sys.path.insert(0, _p)

import numpy as np
import concourse.bass as bass
import concourse.bacc as bacc
import concourse.tile as tile
from concourse import mybir
from concourse.bass_utils import run_bass_kernel_spmd

F32 = mybir.dt.float32
F16 = mybir.dt.float16
EXP = mybir.ActivationFunctionType.Exp

B, N, M, C = 8, 2048, 2048, 512
H, D = 4, 128
SCALE = 1.0 / np.sqrt(C)
P = 128
NB = N // P        # 16 n-blocks
MB = M // P        # 16 m-blocks
KC = C // P        # 4 contraction chunks (also = heads since D=128)
NS = 4             # n-stripes of 512
SW = N // NS       # stripe width 512
JP = MB // 2       # 8 m-block pairs


def build_nc():
    nc = bacc.Bacc(None, target_bir_lowering=False)
    dF1 = nc.dram_tensor("F1", [N, C], F32, kind="ExternalInput")
    dF2 = nc.dram_tensor("F2", [M, C], F32, kind="ExternalInput")
    dW = nc.dram_tensor("Wqkv", [C, C], F32, kind="ExternalInput")
    dBq = nc.dram_tensor("bqkv", [1, C], F32, kind="ExternalInput")
    dWp = nc.dram_tensor("Wproj", [C, C], F32, kind="ExternalInput")
    dBp = nc.dram_tensor("bproj", [1, C], F32, kind="ExternalInput")
    dOut = nc.dram_tensor("OUT", [N, C], F32, kind="ExternalOutput")

    d_ident16 = nc.inline_tensor(np.eye(P, dtype=np.float16), name="identity16")
    d_ones_col = nc.inline_tensor(np.ones((P, 1), np.float16), name="ones_col")

    with tile.TileContext(nc) as tc:
        with (
            tc.tile_pool(name="const", bufs=1) as const,
            tc.tile_pool(name="persist", bufs=1) as persist,
        ):
            # ---- small constants first (cheap, needed early) ----
            ident16 = const.tile([P, P], F16)
            nc.sync.dma_start(ident16, d_ident16[:])
            ones_col = const.tile([P, 1], F16)
            nc.sync.dma_start(ones_col, d_ones_col[:])
            bq_col = const.tile([P, KC], F32)
            nc.sync.dma_start(
                bq_col, dBq[0, :].rearrange("(a b) -> b a", b=P)
            )

            # ---- persistent activations (kc-major packed single tiles) ----
            FT = persist.tile([P, KC, N], F16)    # F^T, reused F2 then F1
            qT = persist.tile([P, KC, N], F16)
            kvT = persist.tile([P, KC, M], F16)
            kvn = persist.tile([P, MB, C], F16)   # kv natural, m-major
            xT = persist.tile([P, KC, N], F16)
            W = []
            for kc in range(KC):
                w = const.tile([P, C], F16, name=f"w{kc}")
                W.append(w)

            # ---- phase 1+2 fused: per-stripe load -> transpose -> proj ----
            with (
                tc.tile_pool(name="fin", bufs=4) as fpool,
                tc.tile_pool(name="ps12", bufs=1, space="PSUM") as ps12,
            ):
                def stripe_dma(dsrc, g):
                    fin = fpool.tile([P, 4, C], F16, tag="fin")
                    nc.gpsimd.dma_start(
                        fin,
                        dsrc[g * SW:(g + 1) * SW, :].rearrange(
                            "(j p) c -> p j c", p=P
                        ),
                    )
                    return fin

                def stripe_tr(fin, g):
                    for half in range(2):
                        tp = ps12.tile([P, 2, SW], F16, tag="tr", bufs=2)
                        for j in range(4):
                            for k in range(2):
                                kc = 2 * half + k
                                nc.tensor.transpose(
                                    tp[:, k, j * P:(j + 1) * P],
                                    fin[:, j, kc * P:(kc + 1) * P],
                                    ident16,
                                )
                        nc.vector.tensor_copy(
                            FT[:, 2 * half:2 * half + 2, g * SW:(g + 1) * SW],
                            tp,
                        )

                def stripe_proj(dst, g):
                    for co in range(KC):
                        pj = ps12.tile([P, SW], F32, tag="pj", bufs=4)
                        for kc in range(KC):
                            nc.tensor.matmul(
                                pj,
                                W[kc][:, co * P:(co + 1) * P],
                                FT[:, kc, g * SW:(g + 1) * SW],
                                start=(kc == 0),
                                stop=(kc == KC - 1),
                            )
                        nc.vector.tensor_scalar_add(
                            dst[:, co, g * SW:(g + 1) * SW],
                            pj,
                            bq_col[:, co:co + 1],
                        )

                # F2: dma g0, W, dma g1, then pipelined tr(g) / proj(g-1)
                fins = {("f2", 0): stripe_dma(dF2, 0)}
                for kc in range(KC):
                    nc.gpsimd.dma_start(W[kc], dW[kc * P:(kc + 1) * P, :])
                fins[("f2", 1)] = stripe_dma(dF2, 1)
                for g in range(NS):
                    if g + 2 < NS:
                        fins[("f2", g + 2)] = stripe_dma(dF2, g + 2)
                    elif g + 2 == NS:  # prefetch F1 stripes behind F2's
                        fins[("f1", 0)] = stripe_dma(dF1, 0)
                        fins[("f1", 1)] = stripe_dma(dF1, 1)
                    stripe_tr(fins[("f2", g)], g)
                    if g > 0:
                        stripe_proj(kvT, g - 1)
                stripe_proj(kvT, NS - 1)

                # kv natural: transpose kv^T blocks (bias already folded)
                for mb in range(MB):
                    pjt = ps12.tile([P, C], F16, tag="tr", bufs=2)
                    for hh in range(H):
                        nc.tensor.transpose(
                            pjt[:, hh * P:(hh + 1) * P],
                            kvT[:, hh, mb * P:(mb + 1) * P],
                            ident16,
                        )
                    nc.vector.tensor_copy(kvn[:, mb, :], pjt)

                # F1 stripes, same pipeline
                for g in range(NS):
                    if g + 2 < NS:
                        fins[("f1", g + 2)] = stripe_dma(dF1, g + 2)
                    stripe_tr(fins[("f1", g)], g)
                    if g > 0:
                        stripe_proj(qT, g - 1)
                stripe_proj(qT, NS - 1)

            # late consts for phase 4 (emitted after input DMAs)
            Wp = []
            for kc in range(KC):
                wp = const.tile([P, C], F16, name=f"wp{kc}")
                nc.gpsimd.dma_start(wp, dWp[kc * P:(kc + 1) * P, :])
                Wp.append(wp)
            bp_row = const.tile([1, C], F32)
            nc.sync.dma_start(bp_row, dBp[:])
            bp_bc = const.tile([P, C], F32)
            nc.gpsimd.partition_broadcast(bp_bc, bp_row)

            # ---- phase 3: attention per (head, n-stripe) ----
            with (
                tc.tile_pool(name="et", bufs=2) as epool,
                tc.tile_pool(name="ps3", bufs=1, space="PSUM") as ps3,
                tc.tile_pool(name="sm", bufs=2) as sm,
            ):
                # deferred normalization state of the previous (h, s) tile
                pend = [None]

                def flush_norm():
                    if pend[0] is None:
                        return
                    ph, ps_, ppv, pdn = pend[0]
                    pend[0] = None
                    rcp = sm.tile([1, SW], F32, tag="rcp")
                    nc.vector.reciprocal_approx_fast(rcp, pdn)
                    rb = sm.tile([P, SW], F32, tag="rb")
                    nc.gpsimd.partition_broadcast(rb, rcp)
                    with nc.allow_low_precision(
                        reason="x values O(0.1); fp16 keeps 5e-4 rel"
                    ):
                        nc.vector.tensor_mul(
                            xT[:, ph, ps_ * SW:(ps_ + 1) * SW], ppv, rb
                        )

                for h in range(H):
                    for s in range(NS):
                        E = epool.tile([P, MB, SW], F16, tag="E")
                        D2 = epool.tile([P, JP, SW], F16, tag="D2")
                        pv = ps3.tile([P, SW], F32, tag="pv", bufs=2)
                        dn = ps3.tile([1, SW], F32, tag="dn", bufs=2)

                        def pv_pair(u):
                            for mb in (2 * u, 2 * u + 1):
                                nc.tensor.matmul(
                                    pv,
                                    kvn[:, mb, h * P:(h + 1) * P],
                                    E[:, mb, :],
                                    start=(mb == 0),
                                    stop=(mb == MB - 1),
                                )

                        def pair_group(k):
                            # D2[2k:2k+2] = E[4k]+E[4k+1], E[4k+2]+E[4k+3]
                            with nc.allow_low_precision(
                                reason="pair sums <=15; fp16 keeps 5e-4 rel"
                            ):
                                nc.vector.tensor_add(
                                    D2[:, 2 * k:2 * k + 2, :],
                                    E[:, 4 * k:4 * k + 4:2, :],
                                    E[:, 4 * k + 1:4 * k + 4:2, :],
                                )

                        def dn_mm(u):
                            nc.tensor.matmul(
                                dn,
                                ones_col,
                                D2[:, u, :],
                                start=(u == 0),
                                stop=(u == JP - 1),
                            )

                        for j in range(JP):
                            sc = ps3.tile([P, 2, SW], F32, tag="sc", bufs=2)
                            for i in range(2):
                                mb = 2 * j + i
                                nc.tensor.matmul(
                                    sc[:, i, :],
                                    kvT[:, h, mb * P:(mb + 1) * P],
                                    qT[:, h, s * SW:(s + 1) * SW],
                                    start=True,
                                    stop=True,
                                )
                            # exp over both banks in one ACT instruction
                            nc.scalar.activation(
                                E[:, 2 * j:2 * j + 2, :].rearrange(
                                    "p a b -> p (a b)"
                                ),
                                sc.rearrange("p a b -> p (a b)"),
                                EXP,
                                scale=float(SCALE),
                            )
                            if j > 0:
                                pv_pair(j - 1)
                            if j == 2:
                                flush_norm()   # prev tile's recip/bcast/mul
                            elif j == 3:
                                pair_group(0)
                            elif j == 5:
                                pair_group(1)
                                dn_mm(0)
                            elif j == 6:
                                dn_mm(1)
                                dn_mm(2)
                            elif j == 7:
                                pair_group(2)
                                dn_mm(3)
                        pv_pair(JP - 1)
                        pair_group(3)
                        for u in range(4, JP):
                            dn_mm(u)
                        pend[0] = (h, s, pv, dn)
                flush_norm()

            # ---- phase 4: output projection ----
            with (
                tc.tile_pool(name="ps4", bufs=1, space="PSUM") as ps4,
                tc.tile_pool(name="osb", bufs=3) as osb,
            ):
                for nb in range(NB):
                    pr = ps4.tile([P, C], F32, tag="pr", bufs=4)
                    for kc in range(KC):
                        nc.tensor.matmul(
                            pr,
                            xT[:, kc, nb * P:(nb + 1) * P],
                            Wp[kc],
                            start=(kc == 0),
                            stop=(kc == KC - 1),
                        )
                    ot = osb.tile([P, C], F32, tag="ot")
                    nc.vector.tensor_add(ot, pr, bp_bc)
                    nc.sync.dma_start(dOut[nb * P:(nb + 1) * P, :], ot)

    nc.compile()
    return nc


_NC = None


def _get_nc():
    global _NC
    if _NC is None:
        _NC = build_nc()
    return _NC


def kernel(F1, F2, W_qkv, b_qkv, W_proj, b_proj, _trace=False):
    F1 = np.ascontiguousarray(np.asarray(F1, dtype=np.float32))
    F2 = np.ascontiguousarray(np.asarray(F2, dtype=np.float32))
    W = np.ascontiguousarray(np.asarray(W_qkv, dtype=np.float32))
    bq = np.ascontiguousarray(np.asarray(b_qkv, dtype=np.float32)).reshape(1, C)
    Wpj = np.ascontiguousarray(np.asarray(W_proj, dtype=np.float32))
    bp = np.ascontiguousarray(np.asarray(b_proj, dtype=np.float32)).reshape(1, C)

    nc = _get_nc()
    in_maps = [
        {"F1": F1[b], "F2": F2[b], "Wqkv": W, "bqkv": bq, "Wproj": Wpj, "bproj": bp}
        for b in range(B)
    ]
    res = run_bass_kernel_spmd(
        nc, in_maps, core_ids=list(range(B)), trace=_trace
    )
    out = np.stack([res.results[b]["OUT"] for b in range(B)], axis=0)
    if _trace:
        return out, res
    return out


# revision 11
# speedup vs baseline: 1.5542x; 1.0158x over previous
"""Trainium2 Bass kernel for nn_CrossAttention (B=8, N=M=2048, C=512, H=4).

Sharding: data-parallel over batch — one batch element per NeuronCore (8 cores).
Per-core dataflow (every matmul contracts over the partition dim; fp16 operands
everywhere -> 1-cycle/row PE; fp32 PSUM accumulation):

  1. Fused input pipeline, software-pipelined per 512-row stripe (F2 then F1):
       casting DMA (gpsimd SWDGE) loads stripe g+2 fp32->fp16 while PE runs
       16 fp16 transposes of stripe g and the projection matmuls of stripe
       g-1 (lhsT = W chunk, rhs = F^T stripe; bias fused into the PSUM->SBUF
       evac as tensor_scalar_add with a per-partition column).
  2. kv natural (m-major) via fp16 PE transposes of kv^T blocks.
  3. Attention per (head, n-stripe of 512), j = pair of m-blocks:
       scores^T[m,n] = kv_h^T.T @ q_h^T   (PSUM, 2 banks per pair)
       E^T = exp(SCALE * scores^T)        (ACT, PSUM->SBUF, fp16; no max-sub:
                                           |SCALE*s| <= ~2, exp safe in fp32)
       unnorm^T[d,n] = sum_m kv_h-blocks.T @ E^T   (PE, fp16 in, f32 acc)
       denominators:  DVE sums E block pairs into D2 (grouped 2 pairs/instr),
                      then 8 PE ones-matmuls (not 16) reduce over partitions;
                      both staggered so PE never waits on the DVE queue
       normalization (recip on the [1,n] row + gpsimd bcast + DVE mul) is
       DEFERRED into the middle of the next (h,s) tile so the slow reciprocal
       never sits ahead of pair-adds in DVE's in-order queue
  4. out[n,c] = x^T-blocks.T @ W_proj (PE) + b_proj via DVE tensor_add with a
     pre-broadcast bias tile (no rank-1 bias matmul, no ACT copy), DMA out.
"""
import sys

for _p in ("/opt/trn_rl_repo", "/root/.axon_site/_ro/trn_rl_repo"):
    if _p not in sys.path:
        sys.path.insert(0, _p)

import numpy as np
import concourse.bass as bass
import concourse.bacc as bacc
import concourse.tile as tile
from concourse import mybir
from concourse.bass_utils import run_bass_kernel_spmd

F32 = mybir.dt.float32
F16 = mybir.dt.float16
EXP = mybir.ActivationFunctionType.Exp

B, N, M, C = 8, 2048, 2048, 512
H, D = 4, 128
SCALE = 1.0 / np.sqrt(C)
P = 128
NB = N // P        # 16 n-blocks
MB = M // P        # 16 m-blocks
KC = C // P        # 4 contraction chunks (also = heads since D=128)
NS = 4             # n-stripes of 512
SW = N // NS       # stripe width 512
JP = MB // 2       # 8 m-block pairs


def build_nc():
    nc = bacc.Bacc(None, target_bir_lowering=False)
    dF1 = nc.dram_tensor("F1", [N, C], F32, kind="ExternalInput")
    dF2 = nc.dram_tensor("F2", [M, C], F32, kind="ExternalInput")
    dW = nc.dram_tensor("Wqkv", [C, C], F32, kind="ExternalInput")
    dBq = nc.dram_tensor("bqkv", [1, C], F32, kind="ExternalInput")
    dWp = nc.dram_tensor("Wproj", [C, C], F32, kind="ExternalInput")
    dBp = nc.dram_tensor("bproj", [1, C], F32, kind="ExternalInput")
    dOut = nc.dram_tensor("OUT", [N, C], F16, kind="ExternalOutput")

    d_ident16 = nc.inline_tensor(np.eye(P, dtype=np.float16), name="identity16")
    d_ones_col = nc.inline_tensor(np.ones((P, 1), np.float16), name="ones_col")

    with tile.TileContext(nc) as tc:
        with (
            tc.tile_pool(name="const", bufs=1) as const,
            tc.tile_pool(name="persist", bufs=1) as persist,
        ):
            # ---- small constants first (cheap, needed early) ----
            ident16 = const.tile([P, P], F16)
            nc.sync.dma_start(ident16, d_ident16[:])
            ones_col = const.tile([P, 1], F16)
            nc.sync.dma_start(ones_col, d_ones_col[:])
            bq_col = const.tile([P, KC], F32)
            nc.sync.dma_start(
                bq_col, dBq[0, :].rearrange("(a b) -> b a", b=P)
            )
            # pre-warm the ACT exp table so the first attention exp doesn't
            # pay the 1.3us ACT_TABLE_LOAD on the critical path
            warm = const.tile([1, 1], F32)
            nc.scalar.activation(warm, bq_col[0:1, 0:1], EXP)

            # ---- persistent activations (kc-major packed single tiles) ----
            FT = persist.tile([P, KC, N], F16)    # F^T, reused F2 then F1
            qT = persist.tile([P, KC, N], F16)
            kvT = persist.tile([P, KC, M], F16)
            kvn = persist.tile([P, MB, C], F16)   # kv natural, m-major
            xT = persist.tile([P, KC, N], F16)
            W = []
            for kc in range(KC):
                w = const.tile([P, C], F16, name=f"w{kc}")
                W.append(w)

            # ---- phase 1+2 fused: per-stripe load -> transpose -> proj ----
            with (
                tc.tile_pool(name="fin", bufs=4) as fpool,
                tc.tile_pool(name="ps12", bufs=1, space="PSUM") as ps12,
            ):
                def stripe_dma(dsrc, g):
                    fin = fpool.tile([P, 4, C], F16, tag="fin")
                    nc.gpsimd.dma_start(
                        fin,
                        dsrc[g * SW:(g + 1) * SW, :].rearrange(
                            "(j p) c -> p j c", p=P
                        ),
                    )
                    return fin

                def stripe_tr(fin, g):
                    for half in range(2):
                        tp = ps12.tile([P, 2, SW], F16, tag="tr", bufs=2)
                        for j in range(4):
                            for k in range(2):
                                kc = 2 * half + k
                                nc.tensor.transpose(
                                    tp[:, k, j * P:(j + 1) * P],
                                    fin[:, j, kc * P:(kc + 1) * P],
                                    ident16,
                                )
                        nc.vector.tensor_copy(
                            FT[:, 2 * half:2 * half + 2, g * SW:(g + 1) * SW],
                            tp,
                        )

                def stripe_proj(dst, g):
                    for co in range(KC):
                        pj = ps12.tile([P, SW], F32, tag="pj", bufs=4)
                        for kc in range(KC):
                            nc.tensor.matmul(
                                pj,
                                W[kc][:, co * P:(co + 1) * P],
                                FT[:, kc, g * SW:(g + 1) * SW],
                                start=(kc == 0),
                                stop=(kc == KC - 1),
                            )
                        nc.vector.tensor_scalar_add(
                            dst[:, co, g * SW:(g + 1) * SW],
                            pj,
                            bq_col[:, co:co + 1],
                        )

                # F2: dma g0, W, dma g1, then pipelined tr(g) / proj(g-1)
                fins = {("f2", 0): stripe_dma(dF2, 0)}
                for kc in range(KC):
                    nc.gpsimd.dma_start(W[kc], dW[kc * P:(kc + 1) * P, :])
                fins[("f2", 1)] = stripe_dma(dF2, 1)
                for g in range(NS):
                    if g + 2 < NS:
                        fins[("f2", g + 2)] = stripe_dma(dF2, g + 2)
                    elif g + 2 == NS:  # prefetch F1 stripes behind F2's
                        fins[("f1", 0)] = stripe_dma(dF1, 0)
                        fins[("f1", 1)] = stripe_dma(dF1, 1)
                    stripe_tr(fins[("f2", g)], g)
                    if g > 0:
                        stripe_proj(kvT, g - 1)
                stripe_proj(kvT, NS - 1)

                # kv natural: transpose kv^T blocks (bias already folded)
                for mb in range(MB):
                    pjt = ps12.tile([P, C], F16, tag="tr", bufs=2)
                    for hh in range(H):
                        nc.tensor.transpose(
                            pjt[:, hh * P:(hh + 1) * P],
                            kvT[:, hh, mb * P:(mb + 1) * P],
                            ident16,
                        )
                    nc.vector.tensor_copy(kvn[:, mb, :], pjt)

                # F1 stripes, same pipeline
                for g in range(NS):
                    if g + 2 < NS:
                        fins[("f1", g + 2)] = stripe_dma(dF1, g + 2)
                    stripe_tr(fins[("f1", g)], g)
                    if g > 0:
                        stripe_proj(qT, g - 1)
                stripe_proj(qT, NS - 1)

            # late consts for phase 4 (emitted after input DMAs)
            Wp = []
            for kc in range(KC):
                wp = const.tile([P, C], F16, name=f"wp{kc}")
                nc.gpsimd.dma_start(wp, dWp[kc * P:(kc + 1) * P, :])
                Wp.append(wp)
            bp_row = const.tile([1, C], F32)
            nc.sync.dma_start(bp_row, dBp[:])
            bp_bc = const.tile([P, C], F32)
            nc.gpsimd.partition_broadcast(bp_bc, bp_row)

            # ---- phase 3: attention per (head, n-stripe) ----
            # Denominator is a 3-level reduction: DVE pair-sums E blocks into
            # D2 (8 blocks), DVE quad-sums D2 into DQ (4 blocks), then 4 PE
            # ones-matmuls. The last two dn matmuls, the reciprocal and the
            # normalization mul of tile T are deferred into tile T+1 so PE
            # never waits on the DVE queue.
            with (
                tc.tile_pool(name="et", bufs=2) as epool,
                tc.tile_pool(name="ps3", bufs=1, space="PSUM") as ps3,
                tc.tile_pool(name="sm", bufs=2) as sm,
            ):
                def dn_mm(dn_, DQ_, v):
                    nc.tensor.matmul(
                        dn_, ones_col, DQ_[:, v, :],
                        start=(v == 0), stop=(v == 3),
                    )

                def dn_mm8(dn_, D2_, u):
                    nc.tensor.matmul(
                        dn_, ones_col, D2_[:, u, :],
                        start=(u == 0), stop=(u == JP - 1),
                    )

                def flush_norm(pn):
                    ph, ps_, ppv, pdn = pn
                    rcp = sm.tile([1, SW], F32, tag="rcp")
                    nc.vector.reciprocal_approx_fast(rcp, pdn)
                    rb = sm.tile([P, SW], F32, tag="rb")
                    nc.gpsimd.partition_broadcast(rb, rcp)
                    with nc.allow_low_precision(
                        reason="x values O(0.1); fp16 keeps 5e-4 rel"
                    ):
                        nc.vector.tensor_mul(
                            xT[:, ph, ps_ * SW:(ps_ + 1) * SW], ppv, rb
                        )

                BISECT_OLD_DN = True
                pend_norm = None        # (h, s, pv, dn) of tile T-1
                pend_dn = None          # (dn, DQ) of tile T-1
                for h in range(H):
                    for s in range(NS):
                        E = epool.tile([P, MB, SW], F16, tag="E")
                        D2 = epool.tile([P, JP, SW], F16, tag="D2")
                        DQ = epool.tile([P, 4, SW], F16, tag="DQ")
                        pv = ps3.tile([P, SW], F32, tag="pv", bufs=2)
                        dn = ps3.tile([1, SW], F32, tag="dn", bufs=2)

                        def pv_pair(u):
                            for mb in (2 * u, 2 * u + 1):
                                nc.tensor.matmul(
                                    pv,
                                    kvn[:, mb, h * P:(h + 1) * P],
                                    E[:, mb, :],
                                    start=(mb == 0),
                                    stop=(mb == MB - 1),
                                )

                        def pair_group(k):
                            # D2[2k:2k+2] = E[4k]+E[4k+1], E[4k+2]+E[4k+3]
                            with nc.allow_low_precision(
                                reason="pair sums <=15; fp16 keeps 5e-4 rel"
                            ):
                                nc.vector.tensor_add(
                                    D2[:, 2 * k:2 * k + 2, :],
                                    E[:, 4 * k:4 * k + 4:2, :],
                                    E[:, 4 * k + 1:4 * k + 4:2, :],
                                )

                        def quad_group(w):
                            # DQ[2w:2w+2] = D2[4w]+D2[4w+1], D2[4w+2]+D2[4w+3]
                            with nc.allow_low_precision(
                                reason="quad sums <=30; fp16 keeps 5e-4 rel"
                            ):
                                nc.vector.tensor_add(
                                    DQ[:, 2 * w:2 * w + 2, :],
                                    D2[:, 4 * w:4 * w + 4:2, :],
                                    D2[:, 4 * w + 1:4 * w + 4:2, :],
                                )

                        for j in range(JP):
                            sc = ps3.tile([P, 2, SW], F32, tag="sc", bufs=2)
                            for i in range(2):
                                mb = 2 * j + i
                                nc.tensor.matmul(
                                    sc[:, i, :],
                                    kvT[:, h, mb * P:(mb + 1) * P],
                                    qT[:, h, s * SW:(s + 1) * SW],
                                    start=True,
                                    stop=True,
                                )
                            # exp over both banks in one ACT instruction
                            nc.scalar.activation(
                                E[:, 2 * j:2 * j + 2, :].rearrange(
                                    "p a b -> p (a b)"
                                ),
                                sc.rearrange("p a b -> p (a b)"),
                                EXP,
                                scale=float(SCALE),
                            )
                            if j > 0:
                                pv_pair(j - 1)
                            if BISECT_OLD_DN:
                                if j == 2:
                                    if pend_norm is not None:
                                        flush_norm(pend_norm)
                                        pend_norm = None
                                elif j == 3:
                                    pair_group(0)
                                elif j == 5:
                                    pair_group(1)
                                    dn_mm8(dn, D2, 0)
                                elif j == 6:
                                    dn_mm8(dn, D2, 1)
                                    dn_mm8(dn, D2, 2)
                                elif j == 7:
                                    pair_group(2)
                                    dn_mm8(dn, D2, 3)
                            else:
                                if j == 1 and pend_dn is not None:
                                    dn_mm(*pend_dn, 2)
                                elif j == 2 and pend_dn is not None:
                                    dn_mm(*pend_dn, 3)
                                    pend_dn = None
                                elif j == 3:
                                    pair_group(0)
                                    if pend_norm is not None:
                                        flush_norm(pend_norm)
                                        pend_norm = None
                                elif j == 5:
                                    pair_group(1)
                                elif j == 6:
                                    quad_group(0)
                                elif j == 7:
                                    pair_group(2)
                                    dn_mm(dn, DQ, 0)
                        pv_pair(JP - 1)
                        pair_group(3)
                        if BISECT_OLD_DN:
                            for u in range(4, JP):
                                dn_mm8(dn, D2, u)
                        else:
                            quad_group(1)
                            dn_mm(dn, DQ, 1)
                            pend_dn = (dn, DQ)
                        pend_norm = (h, s, pv, dn)
                if pend_dn is not None:
                    dn_mm(*pend_dn, 2)
                    dn_mm(*pend_dn, 3)
                flush_norm(pend_norm)

            # ---- phase 4: output projection (fp16 out, 4-block DMA groups
            # so the tail is one 0.5MB transfer, not 16 serialized issues) ----
            with (
                tc.tile_pool(name="ps4", bufs=1, space="PSUM") as ps4,
                tc.tile_pool(name="osb", bufs=2) as osb,
            ):
                for q in range(NB // 4):
                    og = osb.tile([P, 4, C], F16, tag="ot")
                    for jj in range(4):
                        nb = 4 * q + jj
                        pr = ps4.tile([P, C], F32, tag="pr", bufs=4)
                        for kc in range(KC):
                            nc.tensor.matmul(
                                pr,
                                xT[:, kc, nb * P:(nb + 1) * P],
                                Wp[kc],
                                start=(kc == 0),
                                stop=(kc == KC - 1),
                            )
                        with nc.allow_low_precision(
                            reason="out values O(0.1); fp16 keeps 5e-4 rel"
                        ):
                            nc.vector.tensor_add(og[:, jj, :], pr, bp_bc)
                    nc.sync.dma_start(
                        dOut[q * 4 * P:(q + 1) * 4 * P, :].rearrange(
                            "(j p) c -> p j c", p=P
                        ),
                        og,
                    )

    nc.compile()
    return nc


_NC = None


def _get_nc():
    global _NC
    if _NC is None:
        _NC = build_nc()
    return _NC


def kernel(F1, F2, W_qkv, b_qkv, W_proj, b_proj, _trace=False):
    F1 = np.ascontiguousarray(np.asarray(F1, dtype=np.float32))
    F2 = np.ascontiguousarray(np.asarray(F2, dtype=np.float32))
    W = np.ascontiguousarray(np.asarray(W_qkv, dtype=np.float32))
    bq = np.ascontiguousarray(np.asarray(b_qkv, dtype=np.float32)).reshape(1, C)
    Wpj = np.ascontiguousarray(np.asarray(W_proj, dtype=np.float32))
    bp = np.ascontiguousarray(np.asarray(b_proj, dtype=np.float32)).reshape(1, C)

    nc = _get_nc()
    in_maps = [
        {"F1": F1[b], "F2": F2[b], "Wqkv": W, "bqkv": bq, "Wproj": Wpj, "bproj": bp}
        for b in range(B)
    ]
    res = run_bass_kernel_spmd(
        nc, in_maps, core_ids=list(range(B)), trace=_trace
    )
    out = np.stack(
        [np.asarray(res.results[b]["OUT"]).astype(np.float32) for b in range(B)],
        axis=0,
    )
    if _trace:
        return out, res
    return out
